# revision 22
# baseline (speedup 1.0000x reference)
"""MultiHeadAttention Trainium2 Bass kernel.

Problem: B=4, S=2048, E=1024, H=16, Dh=64 (nn_MultiHeadAttention).

Sharding: query-parallel over 8 cores. Core c handles batch b=c//2 and query
rows [qoff, qoff+1024) of that batch (qoff = (c%2)*1024).  Each core computes
K/V for all 2048 keys of its batch (keys are duplicated across the 2 cores of
a batch; 25% extra flops) but needs NO collectives - every core produces its
own 1024 rows of the final output.  Host concatenates.

To keep the program SPMD (single NEFF), the host permutes each core's x so
its query rows are always rows [0, 1024) - attention is invariant to key
order, so the permuted K/V are equivalent.

Per-core kernel math (S=2048 keys, Sq=1024 queries, E=1024, H=16, Dh=64),
all matmuls in bf16 with fp32 PSUM accumulation:
  x_T = transpose(x)                      (DMA xbar transpose, bf16)
  Q_T[hd, sq] = Wq^T x_T + bq             (-> bf16)
  K_T[hd, sk] = Wk^T x_T + bk             (-> bf16)
  V[sk, hd]   = x Wv + bv                 (-> bf16, +ones column)
  S_T[sk, sq] = K_T^T(head slice) Q_T     (per head; h0/h1 on PE row groups
                                           0-63/64-127 run concurrently)
  P_T = exp(S_T / 8)                      (ACT, no max-subtraction: |S|<~9)
  Oa[d+1, sq] = [V|1]^T P_T               (row Dh = softmax denominator)
  O = Oa[:d] * bcast(recip(Oa[d]))        (recip on a [128,8] partition-spread
                                           view via DRAM bounce; broadcast via
                                           stride-0 DRAM DMA)
  y = O_cat Wo + bo
"""

import os
import sys
import types

import numpy as np

B, S, E, H, Dh = 4, 2048, 1024, 16, 64
NCORES = 8
SQ = 1024  # query rows per core
EO = 8  # contraction chunks of 128 over E
NP = 8  # head pairs
KC = 16  # key chunks of 128
QBS = 512  # query block size for the scores matmuls
NQB = SQ // QBS

_cache = {}


def _setup_paths():
    for p in ("/opt/trn_rl_repo", "/root/.axon_site/_ro/trn_rl_repo"):
        if os.path.isdir(p):
            if p not in sys.path:
                sys.path.insert(0, p)
            return


def _install_ntff_hook():
    """Best-effort: register the axon NTFF profile hook so trace=True works."""
    try:
        import antenv

        if "antenv.axon_hooks" in sys.modules:
            return
        hooks_mod = types.ModuleType("antenv.axon_hooks")
        _hook = [None]
        hooks_mod.set_axon_ntff_profile_hook = lambda h: _hook.__setitem__(0, h)
        hooks_mod.get_axon_ntff_profile_hook = lambda: _hook[0]
        sys.modules["antenv.axon_hooks"] = hooks_mod
        antenv.axon_hooks = hooks_mod
        from trn_agent_boot.trn_boot import _ntff_profile_via_ctypes

        so = "/opt/axon/libaxon_pjrt.so"
        if os.path.exists(so):
            hooks_mod.set_axon_ntff_profile_hook(_ntff_profile_via_ctypes(so))
    except Exception:
        pass


def _patch_ldw_opt():
    """Compile NEFFs with --enable-ldw-opt=true: overlapped PE weight loads.

    bass_utils hardcodes false; the flag rewrite happens at the run_command
    layer so it applies to the axon (bass2jax) compile path too.  Results are
    always checked against the reference, so a miscompile would be caught.
    """
    # Disabled: walrus rejects bass-emitted InstLdweights under ldw-opt
    # ("InstLdweights is not compatible with LDW optimization").
    return
    if _cache.get("ldw_patched"):
        return
    from concourse import bass_utils as bu

    orig = bu.run_command

    def patched(argv, **kw):
        argv = [
            "--enable-ldw-opt=true" if a == "--enable-ldw-opt=false" else a
            for a in argv
        ]
        return orig(argv, **kw)

    bu.run_command = patched
    _cache["ldw_patched"] = True


def _build():
    if "nc" in _cache:
        return _cache["nc"]
    _setup_paths()
    _patch_ldw_opt()
    from contextlib import ExitStack

    import concourse.tile as tile
    from concourse import bacc, mybir
    from concourse.bass import ts
    from concourse.masks import make_identity  # noqa: F401

    f32 = mybir.dt.float32
    bf = mybir.dt.bfloat16
    AF = mybir.ActivationFunctionType
    OP = mybir.AluOpType
    scale = 1.0 / float(np.sqrt(Dh))

    nc = bacc.Bacc(None, target_bir_lowering=False)
    xb = nc.dram_tensor("xb", [S, E], bf, kind="ExternalInput")
    wq = nc.dram_tensor("wq", [E, H * Dh], bf, kind="ExternalInput")
    wk = nc.dram_tensor("wk", [E, H * Dh], bf, kind="ExternalInput")
    wv = nc.dram_tensor("wv", [E, H * Dh], bf, kind="ExternalInput")
    wo = nc.dram_tensor("wo", [E, E], bf, kind="ExternalInput")
    bqp = nc.dram_tensor("bqp", [128, NP], f32, kind="ExternalInput")
    bkp = nc.dram_tensor("bkp", [128, NP], f32, kind="ExternalInput")
    bv1 = nc.dram_tensor("bv1", [1, H * Dh], f32, kind="ExternalInput")
    bo1 = nc.dram_tensor("bo1", [1, E], f32, kind="ExternalInput")
    y = nc.dram_tensor("y", [SQ, E], f32, kind="ExternalOutput")

    def echunks(ap):  # [E, F] dram -> [128, EO, F]
        return ap[:, :].rearrange("(eo ei) f -> ei eo f", ei=128)

    with tile.TileContext(nc) as tc, ExitStack() as ctx:
        const = ctx.enter_context(tc.tile_pool(name="const", bufs=1))
        persist = ctx.enter_context(tc.tile_pool(name="persist", bufs=1))

        bq_sb = const.tile([128, NP], f32)
        nc.sync.dma_start(out=bq_sb, in_=bqp[:, :])
        bk_sb = const.tile([128, NP], f32)
        nc.sync.dma_start(out=bk_sb, in_=bkp[:, :])
        bv_bc = const.tile([128, H * Dh], f32)
        nc.sync.dma_start(out=bv_bc, in_=bv1[:, :].to_broadcast((128, H * Dh)))
        bo_bc = const.tile([128, E], f32)
        nc.sync.dma_start(out=bo_bc, in_=bo1[:, :].to_broadcast((128, E)))

        KT = persist.tile([128, NP, S], bf)  # [pair-local hd, pair, sk]
        QT = persist.tile([128, NP, SQ], bf)
        VA = persist.tile([128, KC, H, Dh + 1], bf)  # [sk%128, sk//128, h, d|1]
        OT = persist.tile([128, NP, SQ], bf)

        nc.vector.memset(VA[:, :, :, Dh : Dh + 1], 1.0)

        # ---- interleaved schedule: x transposes first, then each attention
        # pair is preceded by one KQ projection block (and a V block before
        # pairs 0 and 4) so the PE always has dense independent work while
        # ACT grinds through the exps; this keeps the PE HAM clock warm
        # (2.4 GHz) instead of oscillating to 1.2 GHz in exp-wait gaps.
        with tc.tile_pool(name="xtp", bufs=1) as xtp, \
             tc.tile_pool(name="wqkp", bufs=4) as wqkp, \
             tc.tile_pool(name="wvp", bufs=1) as wvp, \
             tc.tile_pool(name="pp", bufs=2) as pp, \
             tc.tile_pool(name="mp", bufs=2) as mp, \
             tc.tile_pool(name="dbp", bufs=2, space="DRAM") as dbp, \
             tc.tile_pool(name="ps_qkv", bufs=2, space="PSUM") as ps_qkv, \
             tc.tile_pool(name="ps_s", bufs=2, space="PSUM") as ps_s, \
             tc.tile_pool(name="ps_o", bufs=2, space="PSUM") as ps_o:

            xT = xtp.tile([128, EO, S], bf)  # [e%128, e//128, s]
            for eo in range(EO):
                nc.sync.dma_start_transpose(
                    out=xT[:, eo, :], in_=xb[:, ts(eo, 128)]
                )

            wq_ch, wk_ch, wv_ch = echunks(wq), echunks(wk), echunks(wv)

            def kq_q(p):
                wq_t = wqkp.tile(
                    [128, EO, 128], bf, tag="wqk", name=f"wq_t{p}"
                )
                nc.sync.dma_start(out=wq_t, in_=wq_ch[:, :, ts(p, 128)])
                for qb in range(NQB):
                    pq = ps_qkv.tile([128, 512], f32, tag="qkv", name=f"pq{p}")
                    for eo in range(EO):
                        nc.tensor.matmul(
                            pq,
                            wq_t[:, eo, :],
                            xT[:, eo, ts(qb, 512)],
                            start=(eo == 0),
                            stop=(eo == EO - 1),
                        )
                    nc.vector.tensor_scalar(
                        out=QT[:, p, ts(qb, 512)],
                        in0=pq,
                        scalar1=bq_sb[:, p : p + 1],
                        scalar2=None,
                        op0=OP.add,
                    )

            def kq_k(p):
                wk_t = wqkp.tile(
                    [128, EO, 128], bf, tag="wqk", name=f"wk_t{p}"
                )
                nc.sync.dma_start(out=wk_t, in_=wk_ch[:, :, ts(p, 128)])
                for kb in range(S // 512):
                    pk = ps_qkv.tile([128, 512], f32, tag="qkv", name=f"pk{p}")
                    for eo in range(EO):
                        nc.tensor.matmul(
                            pk,
                            wk_t[:, eo, :],
                            xT[:, eo, ts(kb, 512)],
                            start=(eo == 0),
                            stop=(eo == EO - 1),
                        )
                    nc.vector.tensor_scalar(
                        out=KT[:, p, ts(kb, 512)],
                        in0=pk,
                        scalar1=bk_sb[:, p : p + 1],
                        scalar2=None,
                        op0=OP.add,
                    )

            def v_block(blk):
                wv_t = wvp.tile(
                    [128, EO, 512], bf, tag="wv", name=f"wv_t{blk}"
                )
                nc.sync.dma_start(out=wv_t, in_=wv_ch[:, :, ts(blk, 512)])
                for skt in range(KC):
                    pv = ps_qkv.tile(
                        [128, 512], f32, tag="qkv", name=f"pv{blk}_{skt}"
                    )
                    for eo in range(EO):
                        nc.tensor.matmul(
                            pv,
                            xT[:, eo, ts(skt, 128)],
                            wv_t[:, eo, :],
                            start=(eo == 0),
                            stop=(eo == EO - 1),
                        )
                    nc.vector.tensor_tensor(
                        out=VA[:, skt, blk * 8 : (blk + 1) * 8, 0:Dh],
                        in0=pv.rearrange("a (h d) -> a h d", d=Dh),
                        in1=bv_bc[:, ts(blk, 512)].rearrange(
                            "a (h d) -> a h d", d=Dh
                        ),
                        op=OP.add,
                    )

            def attention_pair(p, fillers=(None, None)):
                for qb in range(NQB):
                    P_h = [
                        pp.tile(
                            [128, KC, QBS], bf, tag="p", name=f"P{p}_{qb}_0"
                        ),
                        pp.tile(
                            [128, KC, QBS], bf, tag="p", name=f"P{p}_{qb}_1"
                        ),
                    ]
                    for g in range(KC // 2):
                        for hh in range(2):
                            poff = Dh * hh
                            ps = ps_s.tile(
                                [128, 2, QBS], f32, tag="s", name=f"s{p}"
                            )
                            for j in range(2):
                                kc = 2 * g + j
                                nc.tensor.matmul(
                                    ps[:, j, :],
                                    KT[poff : poff + Dh, p, ts(kc, 128)],
                                    QT[poff : poff + Dh, p, ts(qb, QBS)],
                                    start=True,
                                    stop=True,
                                )
                            nc.scalar.activation(
                                out=P_h[hh][:, 2 * g : 2 * g + 2, :],
                                in_=ps,
                                func=AF.Exp,
                                scale=scale,
                            )
                    # independent PE work here fills the exp-lag window so
                    # the AV matmuls below find their P chunks ready.
                    if fillers[qb] is not None:
                        fillers[qb]()
                    for hh in range(2):
                        h = 2 * p + hh
                        poff = Dh * hh
                        po = ps_o.tile(
                            [Dh + 1, QBS], f32, tag="o", name=f"o{p}_{hh}"
                        )
                        for kc in range(KC):
                            nc.tensor.matmul(
                                po,
                                VA[:, kc, h, :],
                                P_h[hh][:, kc, :],
                                start=(kc == 0),
                                stop=(kc == KC - 1),
                            )
                        # softmax denominator: partition-spread reciprocal
                        # via a DRAM bounce, then stride-0 broadcast back.
                        srow = mp.tile([Dh + 1, QBS], f32, tag="srow")
                        nc.vector.tensor_copy(
                            out=srow[Dh : Dh + 1, :], in_=po[Dh : Dh + 1, :]
                        )
                        rcd = dbp.tile([1, QBS], f32, tag="rcd")
                        nc.sync.dma_start(out=rcd, in_=srow[Dh : Dh + 1, :])
                        rsp = mp.tile([128, QBS // 128], f32, tag="rsp")
                        nc.sync.dma_start(
                            out=rsp,
                            in_=rcd[:, :].rearrange(
                                "a (p f) -> p (a f)", p=128
                            ),
                        )
                        rso = mp.tile([128, QBS // 128], f32, tag="rso")
                        nc.vector.reciprocal(out=rso, in_=rsp)
                        rcd2 = dbp.tile([1, QBS], f32, tag="rcd2")
                        nc.sync.dma_start(
                            out=rcd2[:, :].rearrange(
                                "a (p f) -> p (a f)", p=128
                            ),
                            in_=rso,
                        )
                        rb = mp.tile([Dh, QBS], f32, tag="rb")
                        nc.sync.dma_start(
                            out=rb, in_=rcd2[:, :].to_broadcast((Dh, QBS))
                        )
                        nc.vector.tensor_mul(
                            out=OT[poff : poff + Dh, p, ts(qb, QBS)],
                            in0=po[0:Dh, :],
                            in1=rb,
                        )

            kq_k(0)
            kq_q(0)
            v_block(0)
            for p in range(NP):
                if p == 4:
                    v_block(1)
                if p < NP - 1:
                    fl = (lambda q=p + 1: kq_k(q), lambda q=p + 1: kq_q(q))
                else:
                    fl = (None, None)
                attention_pair(p, fillers=fl)

        # ---------------- phase 3: output projection --------------------------
        with tc.tile_pool(name="wop", bufs=1) as wop, \
             tc.tile_pool(name="yp", bufs=3) as yp, \
             tc.tile_pool(name="ps_y", bufs=2, space="PSUM") as ps_y:

            WO = wop.tile([128, NP, E], bf)
            nc.sync.dma_start(
                out=WO, in_=wo[:, :].rearrange("(eo ei) f -> ei eo f", ei=128)
            )

            for stt in range(SQ // 128):
                for nb in range(E // 512):
                    py = ps_y.tile([128, 512], f32, tag="y")
                    for p in range(NP):
                        nc.tensor.matmul(
                            py,
                            OT[:, p, ts(stt, 128)],
                            WO[:, p, ts(nb, 512)],
                            start=(p == 0),
                            stop=(p == NP - 1),
                        )
                    ysb = yp.tile([128, 512], f32, tag="y")
                    nc.vector.tensor_tensor(
                        out=ysb, in0=py, in1=bo_bc[:, ts(nb, 512)], op=OP.add
                    )
                    nc.sync.dma_start(
                        out=y[ts(stt, 128), ts(nb, 512)], in_=ysb
                    )

    nc.finalize()
    _cache["nc"] = nc
    return nc


def _shard_inputs(x, Wq, bq, Wk, bk, Wv, bv, Wo, bo):
    import ml_dtypes

    f32 = np.float32
    bf = ml_dtypes.bfloat16
    wq_t = np.ascontiguousarray(
        np.transpose(np.asarray(Wq, f32), (1, 0, 2)).reshape(E, H * Dh)
    ).astype(bf)
    wk_t = np.ascontiguousarray(
        np.transpose(np.asarray(Wk, f32), (1, 0, 2)).reshape(E, H * Dh)
    ).astype(bf)
    wv_t = np.ascontiguousarray(
        np.transpose(np.asarray(Wv, f32), (1, 0, 2)).reshape(E, H * Dh)
    ).astype(bf)
    wo_b = np.ascontiguousarray(np.asarray(Wo, f32)).astype(bf)
    bq_p = np.ascontiguousarray(np.asarray(bq, f32).reshape(NP, 128).T)
    bk_p = np.ascontiguousarray(np.asarray(bk, f32).reshape(NP, 128).T)
    bv_1 = np.asarray(bv, f32).reshape(1, H * Dh)
    bo_1 = np.asarray(bo, f32).reshape(1, E)

    x = np.asarray(x, f32)
    in_maps = []
    for c in range(NCORES):
        b, qoff = c // 2, (c % 2) * SQ
        if qoff == 0:
            xb_c = x[b]
        else:
            xb_c = np.concatenate([x[b, qoff:], x[b, :qoff]], axis=0)
        in_maps.append(
            {
                "xb": np.ascontiguousarray(xb_c).astype(bf),
                "wq": wq_t,
                "wk": wk_t,
                "wv": wv_t,
                "wo": wo_b,
                "bqp": bq_p,
                "bkp": bk_p,
                "bv1": bv_1,
                "bo1": bo_1,
            }
        )
    return in_maps


def _run(inputs, trace=False, trace_cores=None):
    _setup_paths()
    if trace:
        _install_ntff_hook()
    from concourse.bass_utils import run_bass_kernel_spmd

    nc = _build()
    in_maps = _shard_inputs(**inputs)
    res = run_bass_kernel_spmd(
        nc,
        in_maps,
        core_ids=list(range(NCORES)),
        trace=trace,
        trace_cores=trace_cores,
    )
    out = np.empty((B, S, E), np.float32)
    for c in range(NCORES):
        b, qoff = c // 2, (c % 2) * SQ
        out[b, qoff : qoff + SQ] = res.results[c]["y"]
    return out, res


def kernel(**inputs) -> np.ndarray:
    out, _ = _run(inputs, trace=False)
    return out


# revision 23
# speedup vs baseline: 1.0238x; 1.0238x over previous
"""MultiHeadAttention Trainium2 Bass kernel.

Problem: B=4, S=2048, E=1024, H=16, Dh=64 (nn_MultiHeadAttention).

Sharding: query-parallel over 8 cores. Core c handles batch b=c//2 and query
rows [qoff, qoff+1024) of that batch (qoff = (c%2)*1024).  Each core computes
K/V for all 2048 keys of its batch (keys are duplicated across the 2 cores of
a batch; 25% extra flops) but needs NO collectives - every core produces its
own 1024 rows of the final output.  Host concatenates.

To keep the program SPMD (single NEFF), the host permutes each core's x so
its query rows are always rows [0, 1024) - attention is invariant to key
order, so the permuted K/V are equivalent.

Per-core kernel math (S=2048 keys, Sq=1024 queries, E=1024, H=16, Dh=64),
all matmuls in bf16 with fp32 PSUM accumulation:
  x_T = transpose(x)                      (DMA xbar transpose, bf16)
  Q_T[hd, sq] = Wq^T x_T + bq             (-> bf16)
  K_T[hd, sk] = Wk^T x_T + bk             (-> bf16)
  V[sk, hd]   = x Wv + bv                 (-> bf16, +ones column)
  S_T[sk, sq] = K_T^T(head slice) Q_T     (per head; h0/h1 on PE row groups
                                           0-63/64-127 run concurrently)
  P_T = exp(S_T / 8)                      (ACT, no max-subtraction: |S|<~9)
  Oa[d+1, sq] = [V|1]^T P_T               (row Dh = softmax denominator)
  O = Oa[:d] * bcast(recip(Oa[d]))        (recip on a [128,8] partition-spread
                                           view via DRAM bounce; broadcast via
                                           stride-0 DRAM DMA)
  y = O_cat Wo + bo
"""

import os
import sys
import types

import numpy as np

B, S, E, H, Dh = 4, 2048, 1024, 16, 64
NCORES = 8
SQ = 1024  # query rows per core
EO = 8  # contraction chunks of 128 over E
NP = 8  # head pairs
KC = 16  # key chunks of 128
QBS = 512  # query block size for the scores matmuls
NQB = SQ // QBS

_cache = {}


def _setup_paths():
    for p in ("/opt/trn_rl_repo", "/root/.axon_site/_ro/trn_rl_repo"):
        if os.path.isdir(p):
            if p not in sys.path:
                sys.path.insert(0, p)
            return


def _install_ntff_hook():
    """Best-effort: register the axon NTFF profile hook so trace=True works."""
    try:
        import antenv

        if "antenv.axon_hooks" in sys.modules:
            return
        hooks_mod = types.ModuleType("antenv.axon_hooks")
        _hook = [None]
        hooks_mod.set_axon_ntff_profile_hook = lambda h: _hook.__setitem__(0, h)
        hooks_mod.get_axon_ntff_profile_hook = lambda: _hook[0]
        sys.modules["antenv.axon_hooks"] = hooks_mod
        antenv.axon_hooks = hooks_mod
        from trn_agent_boot.trn_boot import _ntff_profile_via_ctypes

        so = "/opt/axon/libaxon_pjrt.so"
        if os.path.exists(so):
            hooks_mod.set_axon_ntff_profile_hook(_ntff_profile_via_ctypes(so))
    except Exception:
        pass


def _patch_ldw_opt():
    """Compile NEFFs with --enable-ldw-opt=true: overlapped PE weight loads.

    bass_utils hardcodes false; the flag rewrite happens at the run_command
    layer so it applies to the axon (bass2jax) compile path too.  Results are
    always checked against the reference, so a miscompile would be caught.
    """
    # Disabled: walrus rejects bass-emitted InstLdweights under ldw-opt
    # ("InstLdweights is not compatible with LDW optimization").
    return
    if _cache.get("ldw_patched"):
        return
    from concourse import bass_utils as bu

    orig = bu.run_command

    def patched(argv, **kw):
        argv = [
            "--enable-ldw-opt=true" if a == "--enable-ldw-opt=false" else a
            for a in argv
        ]
        return orig(argv, **kw)

    bu.run_command = patched
    _cache["ldw_patched"] = True


def _build():
    if "nc" in _cache:
        return _cache["nc"]
    _setup_paths()
    _patch_ldw_opt()
    from contextlib import ExitStack

    import concourse.tile as tile
    from concourse import bacc, mybir
    from concourse.bass import ts
    from concourse.masks import make_identity  # noqa: F401

    f32 = mybir.dt.float32
    bf = mybir.dt.bfloat16
    AF = mybir.ActivationFunctionType
    OP = mybir.AluOpType
    scale = 1.0 / float(np.sqrt(Dh))

    nc = bacc.Bacc(None, target_bir_lowering=False)
    xb = nc.dram_tensor("xb", [S, E], bf, kind="ExternalInput")
    wq = nc.dram_tensor("wq", [E, H * Dh], bf, kind="ExternalInput")
    wk = nc.dram_tensor("wk", [E, H * Dh], bf, kind="ExternalInput")
    wv = nc.dram_tensor("wv", [E, H * Dh], bf, kind="ExternalInput")
    wo = nc.dram_tensor("wo", [E, E], bf, kind="ExternalInput")
    bqp = nc.dram_tensor("bqp", [128, NP], f32, kind="ExternalInput")
    bkp = nc.dram_tensor("bkp", [128, NP], f32, kind="ExternalInput")
    bv1 = nc.dram_tensor("bv1", [1, H * Dh], f32, kind="ExternalInput")
    bo1 = nc.dram_tensor("bo1", [1, E], f32, kind="ExternalInput")
    y = nc.dram_tensor("y", [SQ, E], f32, kind="ExternalOutput")

    def echunks(ap):  # [E, F] dram -> [128, EO, F]
        return ap[:, :].rearrange("(eo ei) f -> ei eo f", ei=128)

    with tile.TileContext(nc) as tc, ExitStack() as ctx:
        const = ctx.enter_context(tc.tile_pool(name="const", bufs=1))
        persist = ctx.enter_context(tc.tile_pool(name="persist", bufs=1))

        bq_sb = const.tile([128, NP], f32)
        nc.sync.dma_start(out=bq_sb, in_=bqp[:, :])
        bk_sb = const.tile([128, NP], f32)
        nc.sync.dma_start(out=bk_sb, in_=bkp[:, :])
        bv_bc = const.tile([128, H * Dh], f32)
        nc.sync.dma_start(out=bv_bc, in_=bv1[:, :].to_broadcast((128, H * Dh)))
        bo_bc = const.tile([128, E], f32)
        nc.sync.dma_start(out=bo_bc, in_=bo1[:, :].to_broadcast((128, E)))

        KT = persist.tile([128, NP, S], bf)  # [pair-local hd, pair, sk]
        QT = persist.tile([128, NP, SQ], bf)
        VA = persist.tile([128, KC, H, Dh + 1], bf)  # [sk%128, sk//128, h, d|1]
        OT = persist.tile([128, NP, SQ], bf)

        nc.vector.memset(VA[:, :, :, Dh : Dh + 1], 1.0)

        # ---- interleaved schedule: transposes, then KQ/V projection blocks
        # interleaved with per-pair attention so PE always has dense work
        # while ACT grinds through the exps (the attention pacer).
        with tc.tile_pool(name="xtp", bufs=1) as xtp, \
             tc.tile_pool(name="wqkp", bufs=4) as wqkp, \
             tc.tile_pool(name="wvp", bufs=1) as wvp, \
             tc.tile_pool(name="pp", bufs=2) as pp, \
             tc.tile_pool(name="mp", bufs=2) as mp, \
             tc.tile_pool(name="dbp", bufs=2, space="DRAM") as dbp, \
             tc.tile_pool(name="ps_qkv", bufs=2, space="PSUM") as ps_qkv, \
             tc.tile_pool(name="ps_s", bufs=2, space="PSUM") as ps_s, \
             tc.tile_pool(name="ps_o", bufs=2, space="PSUM") as ps_o:

            xT = xtp.tile([128, EO, S], bf)  # [e%128, e//128, s]
            for eo in range(EO):
                nc.sync.dma_start_transpose(
                    out=xT[:, eo, :], in_=xb[:, ts(eo, 128)]
                )

            wq_ch, wk_ch, wv_ch = echunks(wq), echunks(wk), echunks(wv)

            def kq_pair(p):
                wq_t = wqkp.tile(
                    [128, EO, 128], bf, tag="wqk", name=f"wq_t{p}"
                )
                nc.sync.dma_start(out=wq_t, in_=wq_ch[:, :, ts(p, 128)])
                wk_t = wqkp.tile(
                    [128, EO, 128], bf, tag="wqk", name=f"wk_t{p}"
                )
                nc.sync.dma_start(out=wk_t, in_=wk_ch[:, :, ts(p, 128)])
                for qb in range(NQB):
                    pq = ps_qkv.tile([128, 512], f32, tag="qkv", name=f"pq{p}")
                    for eo in range(EO):
                        nc.tensor.matmul(
                            pq,
                            wq_t[:, eo, :],
                            xT[:, eo, ts(qb, 512)],
                            start=(eo == 0),
                            stop=(eo == EO - 1),
                        )
                    nc.vector.tensor_scalar(
                        out=QT[:, p, ts(qb, 512)],
                        in0=pq,
                        scalar1=bq_sb[:, p : p + 1],
                        scalar2=None,
                        op0=OP.add,
                    )
                for kb in range(S // 512):
                    pk = ps_qkv.tile([128, 512], f32, tag="qkv", name=f"pk{p}")
                    for eo in range(EO):
                        nc.tensor.matmul(
                            pk,
                            wk_t[:, eo, :],
                            xT[:, eo, ts(kb, 512)],
                            start=(eo == 0),
                            stop=(eo == EO - 1),
                        )
                    nc.vector.tensor_scalar(
                        out=KT[:, p, ts(kb, 512)],
                        in0=pk,
                        scalar1=bk_sb[:, p : p + 1],
                        scalar2=None,
                        op0=OP.add,
                    )

            def v_block(blk):
                wv_t = wvp.tile(
                    [128, EO, 512], bf, tag="wv", name=f"wv_t{blk}"
                )
                nc.sync.dma_start(out=wv_t, in_=wv_ch[:, :, ts(blk, 512)])
                for skt in range(KC):
                    pv = ps_qkv.tile(
                        [128, 512], f32, tag="qkv", name=f"pv{blk}_{skt}"
                    )
                    for eo in range(EO):
                        nc.tensor.matmul(
                            pv,
                            xT[:, eo, ts(skt, 128)],
                            wv_t[:, eo, :],
                            start=(eo == 0),
                            stop=(eo == EO - 1),
                        )
                    nc.vector.tensor_tensor(
                        out=VA[:, skt, blk * 8 : (blk + 1) * 8, 0:Dh],
                        in0=pv.rearrange("a (h d) -> a h d", d=Dh),
                        in1=bv_bc[:, ts(blk, 512)].rearrange(
                            "a (h d) -> a h d", d=Dh
                        ),
                        op=OP.add,
                    )

            def attention_pair(p):
                for qb in range(NQB):
                    P_h = [
                        pp.tile(
                            [128, KC, QBS], bf, tag="p", name=f"P{p}_{qb}_0"
                        ),
                        pp.tile(
                            [128, KC, QBS], bf, tag="p", name=f"P{p}_{qb}_1"
                        ),
                    ]
                    for g in range(KC // 2):
                        for hh in range(2):
                            poff = Dh * hh
                            ps = ps_s.tile(
                                [128, 2, QBS], f32, tag="s", name=f"s{p}"
                            )
                            for j in range(2):
                                kc = 2 * g + j
                                nc.tensor.matmul(
                                    ps[:, j, :],
                                    KT[poff : poff + Dh, p, ts(kc, 128)],
                                    QT[poff : poff + Dh, p, ts(qb, QBS)],
                                    start=True,
                                    stop=True,
                                )
                            nc.scalar.activation(
                                out=P_h[hh][:, 2 * g : 2 * g + 2, :],
                                in_=ps,
                                func=AF.Exp,
                                scale=scale,
                            )
                    for hh in range(2):
                        h = 2 * p + hh
                        poff = Dh * hh
                        po = ps_o.tile(
                            [Dh + 1, QBS], f32, tag="o", name=f"o{p}_{hh}"
                        )
                        for kc in range(KC):
                            nc.tensor.matmul(
                                po,
                                VA[:, kc, h, :],
                                P_h[hh][:, kc, :],
                                start=(kc == 0),
                                stop=(kc == KC - 1),
                            )
                        # softmax denominator: partition-spread reciprocal
                        # via a DRAM bounce, then stride-0 broadcast back.
                        srow = mp.tile([Dh + 1, QBS], f32, tag="srow")
                        nc.vector.tensor_copy(
                            out=srow[Dh : Dh + 1, :], in_=po[Dh : Dh + 1, :]
                        )
                        rcd = dbp.tile([1, QBS], f32, tag="rcd")
                        nc.sync.dma_start(out=rcd, in_=srow[Dh : Dh + 1, :])
                        rsp = mp.tile([128, QBS // 128], f32, tag="rsp")
                        nc.sync.dma_start(
                            out=rsp,
                            in_=rcd[:, :].rearrange(
                                "a (p f) -> p (a f)", p=128
                            ),
                        )
                        rso = mp.tile([128, QBS // 128], f32, tag="rso")
                        nc.vector.reciprocal(out=rso, in_=rsp)
                        rcd2 = dbp.tile([1, QBS], f32, tag="rcd2")
                        nc.sync.dma_start(
                            out=rcd2[:, :].rearrange(
                                "a (p f) -> p (a f)", p=128
                            ),
                            in_=rso,
                        )
                        rb = mp.tile([Dh, QBS], f32, tag="rb")
                        nc.sync.dma_start(
                            out=rb, in_=rcd2[:, :].to_broadcast((Dh, QBS))
                        )
                        nc.vector.tensor_mul(
                            out=OT[poff : poff + Dh, p, ts(qb, QBS)],
                            in0=po[0:Dh, :],
                            in1=rb,
                        )

            # schedule: prime with 4 KQ pairs + V block 0, then alternate
            # attention pairs with the remaining projection work.
            SCHEDULE = os.environ.get("MHA_SCHEDULE", "interleaved")
            if SCHEDULE == "sequential":
                for p in range(NP):
                    kq_pair(p)
                v_block(0)
                v_block(1)
                for p in range(NP):
                    attention_pair(p)
            else:
                kq_pair(0)
                v_block(0)
                attention_pair(0)
                kq_pair(1)
                attention_pair(1)
                kq_pair(2)
                attention_pair(2)
                kq_pair(3)
                attention_pair(3)
                kq_pair(4)
                v_block(1)
                attention_pair(4)
                kq_pair(5)
                attention_pair(5)
                kq_pair(6)
                attention_pair(6)
                kq_pair(7)
                attention_pair(7)

        # ---------------- phase 3: output projection --------------------------
        with tc.tile_pool(name="wop", bufs=1) as wop, \
             tc.tile_pool(name="yp", bufs=3) as yp, \
             tc.tile_pool(name="ps_y", bufs=2, space="PSUM") as ps_y:

            WO = wop.tile([128, NP, E], bf)
            nc.sync.dma_start(
                out=WO, in_=wo[:, :].rearrange("(eo ei) f -> ei eo f", ei=128)
            )

            for stt in range(SQ // 128):
                for nb in range(E // 512):
                    py = ps_y.tile([128, 512], f32, tag="y")
                    for p in range(NP):
                        nc.tensor.matmul(
                            py,
                            OT[:, p, ts(stt, 128)],
                            WO[:, p, ts(nb, 512)],
                            start=(p == 0),
                            stop=(p == NP - 1),
                        )
                    ysb = yp.tile([128, 512], f32, tag="y")
                    nc.vector.tensor_tensor(
                        out=ysb, in0=py, in1=bo_bc[:, ts(nb, 512)], op=OP.add
                    )
                    nc.sync.dma_start(
                        out=y[ts(stt, 128), ts(nb, 512)], in_=ysb
                    )

    nc.finalize()
    _cache["nc"] = nc
    return nc


def _shard_inputs(x, Wq, bq, Wk, bk, Wv, bv, Wo, bo):
    import ml_dtypes

    f32 = np.float32
    bf = ml_dtypes.bfloat16
    wq_t = np.ascontiguousarray(
        np.transpose(np.asarray(Wq, f32), (1, 0, 2)).reshape(E, H * Dh)
    ).astype(bf)
    wk_t = np.ascontiguousarray(
        np.transpose(np.asarray(Wk, f32), (1, 0, 2)).reshape(E, H * Dh)
    ).astype(bf)
    wv_t = np.ascontiguousarray(
        np.transpose(np.asarray(Wv, f32), (1, 0, 2)).reshape(E, H * Dh)
    ).astype(bf)
    wo_b = np.ascontiguousarray(np.asarray(Wo, f32)).astype(bf)
    bq_p = np.ascontiguousarray(np.asarray(bq, f32).reshape(NP, 128).T)
    bk_p = np.ascontiguousarray(np.asarray(bk, f32).reshape(NP, 128).T)
    bv_1 = np.asarray(bv, f32).reshape(1, H * Dh)
    bo_1 = np.asarray(bo, f32).reshape(1, E)

    x = np.asarray(x, f32)
    in_maps = []
    for c in range(NCORES):
        b, qoff = c // 2, (c % 2) * SQ
        if qoff == 0:
            xb_c = x[b]
        else:
            xb_c = np.concatenate([x[b, qoff:], x[b, :qoff]], axis=0)
        in_maps.append(
            {
                "xb": np.ascontiguousarray(xb_c).astype(bf),
                "wq": wq_t,
                "wk": wk_t,
                "wv": wv_t,
                "wo": wo_b,
                "bqp": bq_p,
                "bkp": bk_p,
                "bv1": bv_1,
                "bo1": bo_1,
            }
        )
    return in_maps


def _run(inputs, trace=False, trace_cores=None):
    _setup_paths()
    if trace:
        _install_ntff_hook()
    from concourse.bass_utils import run_bass_kernel_spmd

    nc = _build()
    in_maps = _shard_inputs(**inputs)
    res = run_bass_kernel_spmd(
        nc,
        in_maps,
        core_ids=list(range(NCORES)),
        trace=trace,
        trace_cores=trace_cores,
    )
    out = np.empty((B, S, E), np.float32)
    for c in range(NCORES):
        b, qoff = c // 2, (c % 2) * SQ
        out[b, qoff : qoff + SQ] = res.results[c]["y"]
    return out, res


def kernel(**inputs) -> np.ndarray:
    out, _ = _run(inputs, trace=False)
    return out
